# revision 11
# baseline (speedup 1.0000x reference)
"""GCN decoder kernel for Trainium2 (8 NeuronCores, data-parallel over batch).

Per batch element b (E=1024 nodes, D=H=768):
  S    = X @ X^T                          (PE, bf16 in / fp32 accum)
  Ahat = sigmoid(S)*m[col] (+I on diag)   (ACT sigmoid + fused DVE mask;
                                           row sums captured by the same op)
  deg  = m * rowsum(Ahat)                 -> dinv = (max(deg,1e-6))^-0.5
  Ahat *= (dinv*m)  per-partition         (folds the left D^-1/2 factor and
                                           the pair mask; A's symmetry makes
                                           the stored [e,f] tile the [f,e] rhs)
  outT = X_lhs.T @ Ahat, scaled by dinv along free dim on PSUM eviction
  HfT  = relu(Wg^T.T @ outT + bg)         (ACT bias+relu eviction)
  PT   = Wp^T.T @ HfT + bp                (DVE bias eviction)
  S2   = PT.T @ PT ; out = sigmoid(S2) * m[row] * m[col]  (fused DVE mask)

dinv also needs to exist as a broadcast row along the free dim; that
transpose is done with a DRAM bounce (contiguous write, strided read) so no
compute engine sits on the critical path, then gpsimd.partition_broadcast.

The gt pool holds one slot per stage-1 chunk (17 > 16), so a whole batch's
sigmoid chain can drain PSUM without waiting on any DVE slot release; that
makes it safe to emit batch 1's stage 1 right after batch 0's (keeping the
PE busy through batch 0's degree chain) without the DVE FIFO priority
inversion deadlocking the schedule.

Sharding: batch 16 -> 2 per core across 8 cores; weights replicated.
"""

import sys

if "/opt/trn_rl_repo" not in sys.path:
    sys.path.insert(0, "/opt/trn_rl_repo")

from contextlib import ExitStack

import numpy as np
import ml_dtypes

import concourse.bass as bass
import concourse.tile as tile
from concourse import bacc, mybir
from concourse.bass_utils import run_bass_kernel_spmd

B, E, D, H = 16, 1024, 768, 768
NCORES = 8
BL = B // NCORES          # batch elements per core
ET = E // 128             # 8 e/f tiles
KD = D // 128             # 6 d/h tiles
FC = E // 512             # 2 moving chunks of 512

FP32 = mybir.dt.float32
BF16 = mybir.dt.bfloat16
AL = mybir.AluOpType
AF = mybir.ActivationFunctionType

_cached_nc = {}


def _build(loops=1):
    if loops in _cached_nc:
        return _cached_nc[loops]

    nc = bacc.Bacc("TRN2", target_bir_lowering=False, debug=False)

    xt_d = nc.dram_tensor("XT", [BL, D, E], BF16, kind="ExternalInput")
    xn_d = nc.dram_tensor("XN", [BL, E, D], BF16, kind="ExternalInput")
    wg_d = nc.dram_tensor("WG", [D, H], BF16, kind="ExternalInput")
    wp_d = nc.dram_tensor("WP", [H, H], BF16, kind="ExternalInput")
    bg_d = nc.dram_tensor("BG", [128, KD], FP32, kind="ExternalInput")
    bp_d = nc.dram_tensor("BP", [128, KD], FP32, kind="ExternalInput")
    mrow_d = nc.dram_tensor("MROW", [BL, 1, E], BF16, kind="ExternalInput")
    mcol_d = nc.dram_tensor("MCOL", [BL, 128, ET], FP32, kind="ExternalInput")
    eye_d = nc.dram_tensor("EYE", [128, 128], BF16, kind="ExternalInput")
    out_d = nc.dram_tensor("OUT", [BL, E, E], FP32, kind="ExternalOutput")

    with tile.TileContext(nc) as tc, ExitStack() as ctx:
        ep = ctx.enter_context
        wpool = ep(tc.tile_pool(name="wpool", bufs=1))
        xtp = ep(tc.tile_pool(name="xt", bufs=2))
        xnp = ep(tc.tile_pool(name="xn", bufs=2))
        ahp = ep(tc.tile_pool(name="ahat", bufs=2))
        rows = ep(tc.tile_pool(name="rows", bufs=2))
        stat = ep(tc.tile_pool(name="stat", bufs=2))
        oftp = ep(tc.tile_pool(name="oft", bufs=1))
        hftp = ep(tc.tile_pool(name="hft", bufs=1))
        ptp = ep(tc.tile_pool(name="ptp", bufs=1))
        rowtmp = ep(tc.tile_pool(name="rowtmp", bufs=1))
        drowp = ep(tc.tile_pool(name="drowp", bufs=2))
        gtp = ep(tc.tile_pool(name="gt", bufs=17))
        ttp = ep(tc.tile_pool(name="tt", bufs=3))
        ostp = ep(tc.tile_pool(name="ost", bufs=2))
        psp = ep(tc.tile_pool(name="psum", bufs=8, space="PSUM"))
        dscr = ep(tc.tile_pool(name="dscr", bufs=2, space="DRAM"))

        # ---- weights / constants (shared across batch) ----
        wg = []
        wp = []
        for k in range(KD):
            t = wpool.tile([128, H], BF16, tag=f"wg{k}")
            nc.sync.dma_start(t[:], wg_d[k * 128:(k + 1) * 128, :])
            wg.append(t)
        for k in range(KD):
            t = wpool.tile([128, H], BF16, tag=f"wp{k}")
            nc.sync.dma_start(t[:], wp_d[k * 128:(k + 1) * 128, :])
            wp.append(t)
        bg = wpool.tile([128, KD], FP32, tag="bg")
        nc.sync.dma_start(bg[:], bg_d[:])
        bp = wpool.tile([128, KD], FP32, tag="bp")
        nc.sync.dma_start(bp[:], bp_d[:])
        eye = wpool.tile([128, 128], BF16, tag="eye")
        nc.sync.dma_start(eye[:], eye_d[:])

        # ---- per-batch inputs ----
        XT = [[None] * KD for _ in range(BL)]
        XN = [[None] * ET for _ in range(BL)]
        MROW = [None] * BL
        MCOL = [None] * BL
        AH = [[None] * ET for _ in range(BL)]
        DROW = [None] * BL
        OFT = [[None] * KD for _ in range(BL)]
        HFT = [[None] * KD for _ in range(BL)]
        PT = [[None] * KD for _ in range(BL)]

        def load_inputs(b):
            for k in range(KD):
                t = xtp.tile([128, E], BF16, tag=f"xt{k}")
                nc.sync.dma_start(t[:], xt_d[b, k * 128:(k + 1) * 128, :])
                XT[b][k] = t
            for k in range(ET):
                t = xnp.tile([128, D], BF16, tag=f"xn{k}")
                nc.sync.dma_start(t[:], xn_d[b, k * 128:(k + 1) * 128, :])
                XN[b][k] = t
            r1 = rows.tile([1, E], BF16, tag="row1")
            nc.sync.dma_start(r1[:], mrow_d[b, :, :])
            mrow = rows.tile([128, E], BF16, tag="mrow")
            nc.gpsimd.partition_broadcast(mrow[:], r1[0:1, :])
            MROW[b] = mrow
            mc = rows.tile([128, ET], FP32, tag="mcol")
            nc.sync.dma_start(mc[:], mcol_d[b, :, :])
            MCOL[b] = mc

        def stage1(b):
            # S = X X^T; Ahat = sigmoid(S) * m[row-free] (+ I); rowsums.
            rs_a = stat.tile([128, ET], FP32, tag="rsa")
            rs_b = stat.tile([128, ET], FP32, tag="rsb")
            for et in range(ET):
                ah = ahp.tile([128, E], BF16, tag=f"ah{et}")
                AH[b][et] = ah
                for fc in range(FC):
                    ps = psp.tile([128, 512], FP32)
                    for k in range(KD):
                        nc.tensor.matmul(
                            ps[:],
                            XT[b][k][:, et * 128:(et + 1) * 128],
                            XT[b][k][:, fc * 512:(fc + 1) * 512],
                            start=(k == 0),
                            stop=(k == KD - 1),
                        )
                    g = gtp.tile([128, 512], BF16)
                    nc.scalar.activation(g[:], ps[:], AF.Sigmoid)
                    rs = (rs_a if fc == 0 else rs_b)
                    nc.vector.scalar_tensor_tensor(
                        out=ah[:, fc * 512:(fc + 1) * 512],
                        in0=g[:],
                        scalar=1.0,
                        in1=MROW[b][:, fc * 512:(fc + 1) * 512],
                        op0=AL.mult,
                        op1=AL.mult,
                        accum_out=rs[:, et:et + 1],
                    )
                # self-loop: +1 on the diagonal 128-block of this e-tile
                nc.vector.tensor_add(
                    ah[:, et * 128:(et + 1) * 128],
                    ah[:, et * 128:(et + 1) * 128],
                    eye[:],
                )

            # degree -> dinv chain (per-partition [128, 8] layout)
            deg = stat.tile([128, ET], FP32, tag="deg")
            nc.vector.tensor_add(deg[:], rs_a[:], rs_b[:])
            nc.vector.scalar_tensor_tensor(
                out=deg[:], in0=deg[:], scalar=1.0, in1=MCOL[b][:],
                op0=AL.add, op1=AL.mult,
            )
            nc.vector.tensor_scalar_max(deg[:], deg[:], 1e-6)
            sq = stat.tile([128, ET], FP32, tag="sq")
            nc.scalar.sqrt(sq[:], deg[:])
            dinv = stat.tile([128, ET], FP32, tag="dinv")
            nc.vector.reciprocal(dinv[:], sq[:])
            dm = stat.tile([128, ET], FP32, tag="dm")
            nc.vector.tensor_mul(dm[:], dinv[:], MCOL[b][:])

            # dinv as a broadcast row: DRAM bounce does the [128,8]->[1,1024]
            # transpose (contiguous write, strided read) off every engine.
            dsc = dscr.tile([128, ET], FP32, tag="dsc")
            nc.sync.dma_start(dsc[:], dinv[:])
            drow1 = rowtmp.tile([1, E], FP32, tag="drow1")
            nc.sync.dma_start(drow1[0:1, :], dsc[:, :].rearrange("p t -> t p"))
            drow = drowp.tile([128, E], FP32, tag="drow")
            nc.gpsimd.partition_broadcast(drow[:], drow1[0:1, :])
            DROW[b] = drow

            # fold dinv[f]*m[f] into stored adjacency (per-partition scale)
            for et in range(ET):
                nc.vector.tensor_scalar_mul(
                    AH[b][et][:], AH[b][et][:], dm[:, et:et + 1]
                )

        def stages2to5(b):
            # -- stage 2: outT[d,e] = sum_f X[f,d] * Ahat[f,e], * dinv[e] --
            for dt in range(KD):
                oft = oftp.tile([128, E], BF16, tag=f"oft{dt}")
                OFT[b][dt] = oft
                for ec in range(FC):
                    ps = psp.tile([128, 512], FP32)
                    for k in range(ET):
                        nc.tensor.matmul(
                            ps[:],
                            XN[b][k][:, dt * 128:(dt + 1) * 128],
                            AH[b][k][:, ec * 512:(ec + 1) * 512],
                            start=(k == 0),
                            stop=(k == ET - 1),
                        )
                    nc.vector.tensor_mul(
                        oft[:, ec * 512:(ec + 1) * 512],
                        ps[:],
                        DROW[b][:, ec * 512:(ec + 1) * 512],
                    )

            # -- stage 3: HfT = relu(Wg.T @ outT + bg) --
            for ht in range(KD):
                hf = hftp.tile([128, E], BF16, tag=f"hft{ht}")
                HFT[b][ht] = hf
                for ec in range(FC):
                    ps = psp.tile([128, 512], FP32)
                    for k in range(KD):
                        nc.tensor.matmul(
                            ps[:],
                            wg[k][:, ht * 128:(ht + 1) * 128],
                            OFT[b][k][:, ec * 512:(ec + 1) * 512],
                            start=(k == 0),
                            stop=(k == KD - 1),
                        )
                    nc.scalar.activation(
                        hf[:, ec * 512:(ec + 1) * 512],
                        ps[:],
                        AF.Relu,
                        bias=bg[:, ht:ht + 1],
                    )

            # -- stage 4: PT = Wp.T @ HfT + bp --
            for ht in range(KD):
                pt = ptp.tile([128, E], BF16, tag=f"pt{ht}")
                PT[b][ht] = pt
                for ec in range(FC):
                    ps = psp.tile([128, 512], FP32)
                    for k in range(KD):
                        nc.tensor.matmul(
                            ps[:],
                            wp[k][:, ht * 128:(ht + 1) * 128],
                            HFT[b][k][:, ec * 512:(ec + 1) * 512],
                            start=(k == 0),
                            stop=(k == KD - 1),
                        )
                    nc.vector.tensor_scalar_add(
                        pt[:, ec * 512:(ec + 1) * 512],
                        ps[:],
                        bp[:, ht:ht + 1],
                    )

            # -- stage 5: Aout = sigmoid(PT.T @ PT) * pair --
            for et in range(ET):
                ost = ostp.tile([128, E], FP32)
                for fc in range(FC):
                    ps = psp.tile([128, 512], FP32)
                    for k in range(KD):
                        nc.tensor.matmul(
                            ps[:],
                            PT[b][k][:, et * 128:(et + 1) * 128],
                            PT[b][k][:, fc * 512:(fc + 1) * 512],
                            start=(k == 0),
                            stop=(k == KD - 1),
                        )
                    t = ttp.tile([128, 512], FP32)
                    nc.scalar.activation(t[:], ps[:], AF.Sigmoid)
                    nc.vector.scalar_tensor_tensor(
                        out=ost[:, fc * 512:(fc + 1) * 512],
                        in0=t[:],
                        scalar=MCOL[b][:, et:et + 1],
                        in1=MROW[b][:, fc * 512:(fc + 1) * 512],
                        op0=AL.mult,
                        op1=AL.mult,
                    )
                nc.sync.dma_start(
                    out_d[b, et * 128:(et + 1) * 128, :], ost[:]
                )

        for b in range(BL):
            load_inputs(b)
        for _ in range(loops):
            for b in range(BL):
                stage1(b)
            for b in range(BL):
                stages2to5(b)

    nc.compile()
    _cached_nc[loops] = nc
    return nc


def make_in_maps(X, mask, W_gcn, b_gcn, W_proj, b_proj):
    bf = ml_dtypes.bfloat16
    X = np.ascontiguousarray(np.asarray(X, dtype=np.float32))
    m = np.asarray(mask).astype(np.float32)
    wg = np.ascontiguousarray(np.asarray(W_gcn, np.float32).T).astype(bf)
    wp = np.ascontiguousarray(np.asarray(W_proj, np.float32).T).astype(bf)
    bg = np.ascontiguousarray(np.asarray(b_gcn, np.float32).reshape(KD, 128).T)
    bp = np.ascontiguousarray(np.asarray(b_proj, np.float32).reshape(KD, 128).T)
    eye = np.eye(128, dtype=bf)
    in_maps = []
    for c in range(NCORES):
        sl = slice(c * BL, (c + 1) * BL)
        Xc = X[sl]
        mc = m[sl]
        in_maps.append({
            "XT": np.ascontiguousarray(Xc.transpose(0, 2, 1)).astype(bf),
            "XN": Xc.astype(bf),
            "WG": wg,
            "WP": wp,
            "BG": bg,
            "BP": bp,
            "MROW": mc.reshape(BL, 1, E).astype(bf),
            "MCOL": np.ascontiguousarray(
                mc.reshape(BL, ET, 128).transpose(0, 2, 1)
            ),
            "EYE": eye,
        })
    return in_maps


def kernel(X, mask, W_gcn, b_gcn, W_proj, b_proj):
    nc = _build()
    in_maps = make_in_maps(X, mask, W_gcn, b_gcn, W_proj, b_proj)
    res = run_bass_kernel_spmd(nc, in_maps, list(range(NCORES)))
    out = np.concatenate([r["OUT"] for r in res.results], axis=0)
    return np.ascontiguousarray(out.astype(np.float32))


# revision 12
# speedup vs baseline: 1.2288x; 1.2288x over previous
"""GCN decoder kernel for Trainium2 (8 NeuronCores, data-parallel over batch).

Per batch element b (E=1024 nodes, D=H=768):
  S    = X @ X^T                          (PE, bf16 in / fp32 accum)
  Ahat = sigmoid(S)*m[col] (+I on diag)   (ACT sigmoid + fused DVE mask;
                                           row sums captured by the same op)
  deg  = m * rowsum(Ahat)                 -> dinv = (max(deg,1e-6))^-0.5
  Ahat *= (dinv*m)  per-partition         (folds the left D^-1/2 factor and
                                           the pair mask; A's symmetry makes
                                           the stored [e,f] tile the [f,e] rhs)
  outT = X_lhs.T @ Ahat, scaled by dinv along free dim on PSUM eviction
  HfT  = relu(Wg^T.T @ outT + bg)         (ACT bias+relu eviction)
  PT   = Wp^T.T @ HfT + bp                (DVE bias eviction)
  S2   = PT.T @ PT ; out = sigmoid(S2) * m[row] * m[col]  (fused DVE mask)

dinv also needs to exist as a broadcast row along the free dim; that
transpose is done with a DRAM bounce (contiguous write, strided read) so no
compute engine sits on the critical path, then gpsimd.partition_broadcast.

The gt pool holds one slot per stage-1 chunk (17 > 16), so a whole batch's
sigmoid chain can drain PSUM without waiting on any DVE slot release; that
makes it safe to emit batch 1's stage 1 right after batch 0's (keeping the
PE busy through batch 0's degree chain) without the DVE FIFO priority
inversion deadlocking the schedule.

Sharding: batch 16 -> 2 per core across 8 cores; weights replicated.
"""

import sys

if "/opt/trn_rl_repo" not in sys.path:
    sys.path.insert(0, "/opt/trn_rl_repo")

from contextlib import ExitStack

import numpy as np
import ml_dtypes

import concourse.bass as bass
import concourse.tile as tile
from concourse import bacc, mybir
from concourse.bass_utils import run_bass_kernel_spmd

B, E, D, H = 16, 1024, 768, 768
NCORES = 8
BL = B // NCORES          # batch elements per core
ET = E // 128             # 8 e/f tiles
KD = D // 128             # 6 d/h tiles
FC = E // 512             # 2 moving chunks of 512

FP32 = mybir.dt.float32
BF16 = mybir.dt.bfloat16
FP8 = mybir.dt.float8e4
KP = D // 256             # 3 fp8 DoubleRow contraction pair-tiles
AL = mybir.AluOpType
AF = mybir.ActivationFunctionType

_cached_nc = {}


def _build(loops=1):
    if loops in _cached_nc:
        return _cached_nc[loops]

    nc = bacc.Bacc("TRN2", target_bir_lowering=False, debug=False)

    xp_d = nc.dram_tensor("XP", [BL, KP, 128, 2, E], FP8, kind="ExternalInput")
    xn_d = nc.dram_tensor("XN", [BL, E, D], BF16, kind="ExternalInput")
    wg_d = nc.dram_tensor("WG", [D, H], BF16, kind="ExternalInput")
    wp_d = nc.dram_tensor("WP", [H, H], BF16, kind="ExternalInput")
    bg_d = nc.dram_tensor("BG", [128, KD], FP32, kind="ExternalInput")
    bp_d = nc.dram_tensor("BP", [128, KD], FP32, kind="ExternalInput")
    mrow_d = nc.dram_tensor("MROW", [BL, 1, E], BF16, kind="ExternalInput")
    mcol_d = nc.dram_tensor("MCOL", [BL, 128, ET], FP32, kind="ExternalInput")
    eye_d = nc.dram_tensor("EYE", [128, 128], BF16, kind="ExternalInput")
    out_d = nc.dram_tensor("OUT", [BL, E, E], FP32, kind="ExternalOutput")

    with tile.TileContext(nc) as tc, ExitStack() as ctx:
        ep = ctx.enter_context
        wpool = ep(tc.tile_pool(name="wpool", bufs=1))
        xtp = ep(tc.tile_pool(name="xt", bufs=2))
        xnp = ep(tc.tile_pool(name="xn", bufs=2))
        ahp = ep(tc.tile_pool(name="ahat", bufs=2))
        rows = ep(tc.tile_pool(name="rows", bufs=2))
        stat = ep(tc.tile_pool(name="stat", bufs=2))
        oftp = ep(tc.tile_pool(name="oft", bufs=1))
        hftp = ep(tc.tile_pool(name="hft", bufs=1))
        ptp = ep(tc.tile_pool(name="ptp", bufs=1))
        rowtmp = ep(tc.tile_pool(name="rowtmp", bufs=1))
        drowp = ep(tc.tile_pool(name="drowp", bufs=2))
        gtp = ep(tc.tile_pool(name="gt", bufs=17))
        ttp = ep(tc.tile_pool(name="tt", bufs=3))
        ostp = ep(tc.tile_pool(name="ost", bufs=2))
        psp = ep(tc.tile_pool(name="psum", bufs=8, space="PSUM"))
        dscr = ep(tc.tile_pool(name="dscr", bufs=2, space="DRAM"))

        # ---- weights / constants (shared across batch) ----
        wg = []
        wp = []
        for k in range(KD):
            t = wpool.tile([128, H], BF16, tag=f"wg{k}")
            nc.sync.dma_start(t[:], wg_d[k * 128:(k + 1) * 128, :])
            wg.append(t)
        for k in range(KD):
            t = wpool.tile([128, H], BF16, tag=f"wp{k}")
            nc.sync.dma_start(t[:], wp_d[k * 128:(k + 1) * 128, :])
            wp.append(t)
        bg = wpool.tile([128, KD], FP32, tag="bg")
        nc.sync.dma_start(bg[:], bg_d[:])
        bp = wpool.tile([128, KD], FP32, tag="bp")
        nc.sync.dma_start(bp[:], bp_d[:])
        eye = wpool.tile([128, 128], BF16, tag="eye")
        nc.sync.dma_start(eye[:], eye_d[:])

        # ---- per-batch inputs ----
        XT = [[None] * KP for _ in range(BL)]
        XN = [[None] * ET for _ in range(BL)]
        MROW = [None] * BL
        MCOL = [None] * BL
        AH = [[None] * ET for _ in range(BL)]
        DROW = [None] * BL
        OFT = [[None] * KD for _ in range(BL)]
        HFT = [[None] * KD for _ in range(BL)]
        PT = [[None] * KP for _ in range(BL)]

        def load_inputs(b):
            for k in range(KP):
                t = xtp.tile([128, 2, E], FP8, tag=f"xp{k}")
                nc.sync.dma_start(t[:], xp_d[b, k, :, :, :])
                XT[b][k] = t
            for k in range(ET):
                t = xnp.tile([128, D], BF16, tag=f"xn{k}")
                nc.sync.dma_start(t[:], xn_d[b, k * 128:(k + 1) * 128, :])
                XN[b][k] = t
            r1 = rows.tile([1, E], BF16, tag="row1")
            nc.sync.dma_start(r1[:], mrow_d[b, :, :])
            mrow = rows.tile([128, E], BF16, tag="mrow")
            nc.gpsimd.partition_broadcast(mrow[:], r1[0:1, :])
            MROW[b] = mrow
            mc = rows.tile([128, ET], FP32, tag="mcol")
            nc.sync.dma_start(mc[:], mcol_d[b, :, :])
            MCOL[b] = mc

        def stage1(b):
            # S = X X^T; Ahat = sigmoid(S) * m[row-free] (+ I); rowsums.
            rs_a = stat.tile([128, ET], FP32, tag="rsa")
            rs_b = stat.tile([128, ET], FP32, tag="rsb")
            for et in range(ET):
                ah = ahp.tile([128, E], BF16, tag=f"ah{et}")
                AH[b][et] = ah
                for fc in range(FC):
                    ps = psp.tile([128, 512], FP32)
                    for k in range(KP):
                        nc.tensor.matmul(
                            ps[:],
                            XT[b][k][:, :, et * 128:(et + 1) * 128],
                            XT[b][k][:, :, fc * 512:(fc + 1) * 512],
                            start=(k == 0),
                            stop=(k == KP - 1),
                            perf_mode=mybir.MatmulPerfMode.DoubleRow,
                        )
                    g = gtp.tile([128, 512], BF16)
                    nc.scalar.activation(g[:], ps[:], AF.Sigmoid)
                    rs = (rs_a if fc == 0 else rs_b)
                    nc.vector.scalar_tensor_tensor(
                        out=ah[:, fc * 512:(fc + 1) * 512],
                        in0=g[:],
                        scalar=1.0,
                        in1=MROW[b][:, fc * 512:(fc + 1) * 512],
                        op0=AL.mult,
                        op1=AL.mult,
                        accum_out=rs[:, et:et + 1],
                    )
                # self-loop: +1 on the diagonal 128-block of this e-tile
                nc.vector.tensor_add(
                    ah[:, et * 128:(et + 1) * 128],
                    ah[:, et * 128:(et + 1) * 128],
                    eye[:],
                )

            # degree -> dinv chain (per-partition [128, 8] layout)
            deg = stat.tile([128, ET], FP32, tag="deg")
            nc.vector.tensor_add(deg[:], rs_a[:], rs_b[:])
            nc.vector.scalar_tensor_tensor(
                out=deg[:], in0=deg[:], scalar=1.0, in1=MCOL[b][:],
                op0=AL.add, op1=AL.mult,
            )
            nc.vector.tensor_scalar_max(deg[:], deg[:], 1e-6)
            sq = stat.tile([128, ET], FP32, tag="sq")
            nc.scalar.sqrt(sq[:], deg[:])
            dinv = stat.tile([128, ET], FP32, tag="dinv")
            nc.vector.reciprocal(dinv[:], sq[:])
            dm = stat.tile([128, ET], FP32, tag="dm")
            nc.vector.tensor_mul(dm[:], dinv[:], MCOL[b][:])

            # dinv as a broadcast row: DRAM bounce does the [128,8]->[1,1024]
            # transpose (contiguous write, strided read) off every engine.
            dsc = dscr.tile([128, ET], FP32, tag="dsc")
            nc.sync.dma_start(dsc[:], dinv[:])
            drow1 = rowtmp.tile([1, E], FP32, tag="drow1")
            nc.sync.dma_start(drow1[0:1, :], dsc[:, :].rearrange("p t -> t p"))
            drow = drowp.tile([128, E], FP32, tag="drow")
            nc.gpsimd.partition_broadcast(drow[:], drow1[0:1, :])
            DROW[b] = drow

            # fold dinv[f]*m[f] into stored adjacency (per-partition scale)
            for et in range(ET):
                nc.vector.tensor_scalar_mul(
                    AH[b][et][:], AH[b][et][:], dm[:, et:et + 1]
                )

        def stages2to5(b):
            # -- stage 2: outT[d,e] = sum_f X[f,d] * Ahat[f,e], * dinv[e] --
            for dt in range(KD):
                oft = oftp.tile([128, E], BF16, tag=f"oft{dt}")
                OFT[b][dt] = oft
                for ec in range(FC):
                    ps = psp.tile([128, 512], FP32)
                    for k in range(ET):
                        nc.tensor.matmul(
                            ps[:],
                            XN[b][k][:, dt * 128:(dt + 1) * 128],
                            AH[b][k][:, ec * 512:(ec + 1) * 512],
                            start=(k == 0),
                            stop=(k == ET - 1),
                        )
                    nc.vector.tensor_mul(
                        oft[:, ec * 512:(ec + 1) * 512],
                        ps[:],
                        DROW[b][:, ec * 512:(ec + 1) * 512],
                    )

            # -- stage 3: HfT = relu(Wg.T @ outT + bg) --
            for ht in range(KD):
                hf = hftp.tile([128, E], BF16, tag=f"hft{ht}")
                HFT[b][ht] = hf
                for ec in range(FC):
                    ps = psp.tile([128, 512], FP32)
                    for k in range(KD):
                        nc.tensor.matmul(
                            ps[:],
                            wg[k][:, ht * 128:(ht + 1) * 128],
                            OFT[b][k][:, ec * 512:(ec + 1) * 512],
                            start=(k == 0),
                            stop=(k == KD - 1),
                        )
                    nc.scalar.activation(
                        hf[:, ec * 512:(ec + 1) * 512],
                        ps[:],
                        AF.Relu,
                        bias=bg[:, ht:ht + 1],
                    )

            # -- stage 4: PT = Wp.T @ HfT + bp (evicted as fp8 pair tiles) --
            for ht in range(KD):
                if ht % 2 == 0:
                    pt = ptp.tile([128, 2, E], FP8, tag=f"pt{ht // 2}")
                    PT[b][ht // 2] = pt
                else:
                    pt = PT[b][ht // 2]
                for ec in range(FC):
                    ps = psp.tile([128, 512], FP32)
                    for k in range(KD):
                        nc.tensor.matmul(
                            ps[:],
                            wp[k][:, ht * 128:(ht + 1) * 128],
                            HFT[b][k][:, ec * 512:(ec + 1) * 512],
                            start=(k == 0),
                            stop=(k == KD - 1),
                        )
                    nc.vector.tensor_scalar_add(
                        pt[:, ht % 2, ec * 512:(ec + 1) * 512],
                        ps[:],
                        bp[:, ht:ht + 1],
                    )

            # -- stage 5: Aout = sigmoid(PT.T @ PT) * pair --
            for et in range(ET):
                ost = ostp.tile([128, E], FP32)
                for fc in range(FC):
                    ps = psp.tile([128, 512], FP32)
                    for k in range(KP):
                        nc.tensor.matmul(
                            ps[:],
                            PT[b][k][:, :, et * 128:(et + 1) * 128],
                            PT[b][k][:, :, fc * 512:(fc + 1) * 512],
                            start=(k == 0),
                            stop=(k == KP - 1),
                            perf_mode=mybir.MatmulPerfMode.DoubleRow,
                        )
                    t = ttp.tile([128, 512], FP32)
                    nc.scalar.activation(t[:], ps[:], AF.Sigmoid)
                    nc.vector.scalar_tensor_tensor(
                        out=ost[:, fc * 512:(fc + 1) * 512],
                        in0=t[:],
                        scalar=MCOL[b][:, et:et + 1],
                        in1=MROW[b][:, fc * 512:(fc + 1) * 512],
                        op0=AL.mult,
                        op1=AL.mult,
                    )
                nc.sync.dma_start(
                    out_d[b, et * 128:(et + 1) * 128, :], ost[:]
                )

        for b in range(BL):
            load_inputs(b)
        for _ in range(loops):
            for b in range(BL):
                stage1(b)
            for b in range(BL):
                stages2to5(b)

    nc.compile()
    _cached_nc[loops] = nc
    return nc


def make_in_maps(X, mask, W_gcn, b_gcn, W_proj, b_proj):
    bf = ml_dtypes.bfloat16
    f8 = mybir.dt.np(FP8)
    X = np.ascontiguousarray(np.asarray(X, dtype=np.float32))
    m = np.asarray(mask).astype(np.float32)
    wg = np.ascontiguousarray(np.asarray(W_gcn, np.float32).T).astype(bf)
    wp = np.ascontiguousarray(np.asarray(W_proj, np.float32).T).astype(bf)
    bg = np.ascontiguousarray(np.asarray(b_gcn, np.float32).reshape(KD, 128).T)
    bp = np.ascontiguousarray(np.asarray(b_proj, np.float32).reshape(KD, 128).T)
    eye = np.eye(128, dtype=bf)
    in_maps = []
    for c in range(NCORES):
        sl = slice(c * BL, (c + 1) * BL)
        Xc = X[sl]
        mc = m[sl]
        in_maps.append({
            "XP": np.ascontiguousarray(
                Xc.transpose(0, 2, 1).reshape(BL, KP, 2, 128, E)
                .transpose(0, 1, 3, 2, 4)
            ).astype(f8),
            "XN": Xc.astype(bf),
            "WG": wg,
            "WP": wp,
            "BG": bg,
            "BP": bp,
            "MROW": mc.reshape(BL, 1, E).astype(bf),
            "MCOL": np.ascontiguousarray(
                mc.reshape(BL, ET, 128).transpose(0, 2, 1)
            ),
            "EYE": eye,
        })
    return in_maps


def kernel(X, mask, W_gcn, b_gcn, W_proj, b_proj):
    nc = _build()
    in_maps = make_in_maps(X, mask, W_gcn, b_gcn, W_proj, b_proj)
    res = run_bass_kernel_spmd(nc, in_maps, list(range(NCORES)))
    out = np.concatenate([r["OUT"] for r in res.results], axis=0)
    return np.ascontiguousarray(out.astype(np.float32))


# revision 20
# speedup vs baseline: 1.5768x; 1.2832x over previous
"""GCN decoder kernel for Trainium2 (8 NeuronCores, data-parallel over batch).

Per batch element b (E=1024 nodes, D=H=768):
  S    = X @ X^T                          (PE, bf16 in / fp32 accum)
  Ahat = sigmoid(S)*m[col] (+I on diag)   (ACT sigmoid + fused DVE mask;
                                           row sums captured by the same op)
  deg  = m * rowsum(Ahat)                 -> dinv = (max(deg,1e-6))^-0.5
  Ahat *= (dinv*m)  per-partition         (folds the left D^-1/2 factor and
                                           the pair mask; A's symmetry makes
                                           the stored [e,f] tile the [f,e] rhs)
  outT = X_lhs.T @ Ahat, scaled by dinv along free dim on PSUM eviction
  HfT  = relu(Wg^T.T @ outT + bg)         (ACT bias+relu eviction)
  PT   = Wp^T.T @ HfT + bp                (DVE bias eviction)
  S2   = PT.T @ PT ; out = sigmoid(S2) * m[row] * m[col]  (fused DVE mask)

dinv also needs to exist as a broadcast row along the free dim; that
transpose is done with a DRAM bounce (contiguous write, strided read) so no
compute engine sits on the critical path, then gpsimd.partition_broadcast.

The gt pool holds one slot per stage-1 chunk (17 > 16), so a whole batch's
sigmoid chain can drain PSUM without waiting on any DVE slot release; that
makes it safe to emit batch 1's stage 1 right after batch 0's (keeping the
PE busy through batch 0's degree chain) without the DVE FIFO priority
inversion deadlocking the schedule.

Sharding: batch 16 -> 2 per core across 8 cores; weights replicated.
"""

import sys

if "/opt/trn_rl_repo" not in sys.path:
    sys.path.insert(0, "/opt/trn_rl_repo")

from contextlib import ExitStack

import numpy as np
import ml_dtypes

import concourse.bass as bass
import concourse.tile as tile
from concourse import bacc, mybir
from concourse.bass_utils import run_bass_kernel_spmd

B, E, D, H = 16, 1024, 768, 768
NCORES = 8
BL = B // NCORES          # batch elements per core
ET = E // 128             # 8 e/f tiles
KD = D // 128             # 6 d/h tiles
FC = E // 512             # 2 moving chunks of 512

FP32 = mybir.dt.float32
BF16 = mybir.dt.bfloat16
FP8 = mybir.dt.float8e4
KP = D // 256             # 3 fp8 DoubleRow contraction pair-tiles
AL = mybir.AluOpType
AF = mybir.ActivationFunctionType

_cached_nc = {}


def _build(loops=1):
    if loops in _cached_nc:
        return _cached_nc[loops]

    nc = bacc.Bacc("TRN2", target_bir_lowering=False, debug=False)

    xp_d = nc.dram_tensor("XP", [BL, KP, 128, 2, E], FP8, kind="ExternalInput")
    xn_d = nc.dram_tensor("XN", [BL, E, D], BF16, kind="ExternalInput")
    wg_d = nc.dram_tensor("WG", [KP, 128, 2, H], FP8, kind="ExternalInput")
    wp_d = nc.dram_tensor("WP", [KP, 128, 2, H], FP8, kind="ExternalInput")
    bg_d = nc.dram_tensor("BG", [128, KD], FP32, kind="ExternalInput")
    bp_d = nc.dram_tensor("BP", [128, KD], FP32, kind="ExternalInput")
    mrow_d = nc.dram_tensor("MROW", [BL, 1, E], BF16, kind="ExternalInput")
    mcol_d = nc.dram_tensor("MCOL", [BL, 128, ET], FP32, kind="ExternalInput")
    eye_d = nc.dram_tensor("EYE", [128, 128], BF16, kind="ExternalInput")
    out_d = nc.dram_tensor("OUT", [BL, E, E], FP32, kind="ExternalOutput")

    with tile.TileContext(nc) as tc, ExitStack() as ctx:
        ep = ctx.enter_context
        wpool = ep(tc.tile_pool(name="wpool", bufs=1))
        xtp = ep(tc.tile_pool(name="xt", bufs=2))
        xnp = ep(tc.tile_pool(name="xn", bufs=2))
        ahp = ep(tc.tile_pool(name="ahat", bufs=2))
        rows = ep(tc.tile_pool(name="rows", bufs=2))
        stat = ep(tc.tile_pool(name="stat", bufs=2))
        oftp = ep(tc.tile_pool(name="oft", bufs=2))
        hftp = ep(tc.tile_pool(name="hft", bufs=2))
        ptp = ep(tc.tile_pool(name="ptp", bufs=2))
        rowtmp = ep(tc.tile_pool(name="rowtmp", bufs=1))
        drowp = ep(tc.tile_pool(name="drowp", bufs=2))
        gtp = ep(tc.tile_pool(name="gt", bufs=9))
        ostp = ep(tc.tile_pool(name="ost", bufs=4))
        psA = ep(tc.tile_pool(name="psumA", bufs=2, space="PSUM"))
        psB = ep(tc.tile_pool(name="psumB", bufs=2, space="PSUM"))
        dscr = ep(tc.tile_pool(name="dscr", bufs=2, space="DRAM"))

        # ---- per-batch inputs ----
        XT = [[None] * KP for _ in range(BL)]
        XN = [[None] * ET for _ in range(BL)]
        MROW = [None] * BL
        MCOL = [None] * BL
        AH = [[None] * ET for _ in range(BL)]
        DROW = [None] * BL
        OFT = [[None] * KP for _ in range(BL)]
        HFT = [[None] * KP for _ in range(BL)]
        PT = [[None] * KP for _ in range(BL)]

        eye = wpool.tile([128, 128], BF16, tag="eye")
        nc.sync.dma_start(eye[:], eye_d[:])

        def load_inputs(b):
            for k in range(KP):
                t = xtp.tile([128, 2, E], FP8, tag=f"xp{k}")
                nc.sync.dma_start(t[:, :, 0:512], xp_d[b, k, :, :, 0:512])
                nc.sync.dma_start(t[:, :, 512:E], xp_d[b, k, :, :, 512:E])
                XT[b][k] = t
            r1 = rows.tile([1, E], BF16, tag="row1")
            nc.sync.dma_start(r1[:], mrow_d[b, :, :])
            mrow = rows.tile([128, E], BF16, tag="mrow")
            nc.gpsimd.partition_broadcast(mrow[:], r1[0:1, :])
            MROW[b] = mrow
            mc = rows.tile([128, ET], FP32, tag="mcol")
            nc.sync.dma_start(mc[:], mcol_d[b, :, :])
            MCOL[b] = mc

        PSP = [None, None]

        def stage1(b):
            psp = PSP[b]
            # S = X X^T; Ahat = sigmoid(S) * m[row-free] (+ I); rowsums.
            rs_a = stat.tile([128, ET], FP32, tag="rsa")
            for et in range(ET):
                ah = ahp.tile([128, E], BF16, tag=f"ah{et}")
                AH[b][et] = ah
                ps = psp.tile([128, E], FP32)
                for fc in range(FC):
                    for k in range(KP):
                        nc.tensor.matmul(
                            ps[:, fc * 512:(fc + 1) * 512],
                            XT[b][k][:, :, et * 128:(et + 1) * 128],
                            XT[b][k][:, :, fc * 512:(fc + 1) * 512],
                            start=(k == 0),
                            stop=(k == KP - 1),
                            perf_mode=mybir.MatmulPerfMode.DoubleRow,
                        )
                g = gtp.tile([128, E], BF16)
                nc.scalar.activation(g[:], ps[:], AF.Sigmoid)
                nc.vector.scalar_tensor_tensor(
                    out=ah[:],
                    in0=g[:],
                    scalar=1.0,
                    in1=MROW[b][:],
                    op0=AL.mult,
                    op1=AL.mult,
                    accum_out=rs_a[:, et:et + 1],
                )
                # self-loop: +1 on the diagonal 128-block of this e-tile
                nc.vector.tensor_add(
                    ah[:, et * 128:(et + 1) * 128],
                    ah[:, et * 128:(et + 1) * 128],
                    eye[:],
                )

            # degree -> dinv chain (per-partition [128, 8] layout)
            deg = stat.tile([128, ET], FP32, tag="deg")
            nc.vector.scalar_tensor_tensor(
                out=deg[:], in0=rs_a[:], scalar=1.0, in1=MCOL[b][:],
                op0=AL.add, op1=AL.mult,
            )
            nc.vector.tensor_scalar_max(deg[:], deg[:], 1e-6)
            sq = stat.tile([128, ET], FP32, tag="sq")
            nc.scalar.sqrt(sq[:], deg[:])
            dinv = stat.tile([128, ET], FP32, tag="dinv")
            nc.vector.reciprocal(dinv[:], sq[:])
            dm = stat.tile([128, ET], FP32, tag="dm")
            nc.vector.tensor_mul(dm[:], dinv[:], MCOL[b][:])

            # dinv as a broadcast row: DRAM bounce does the [128,8]->[1,1024]
            # transpose (contiguous write, strided read) off every engine.
            dsc = dscr.tile([128, ET], FP32, tag="dsc")
            nc.sync.dma_start(dsc[:], dinv[:])
            drow1 = rowtmp.tile([1, E], FP32, tag="drow1")
            nc.sync.dma_start(drow1[0:1, :], dsc[:, :].rearrange("p t -> t p"))
            drow = drowp.tile([128, E], FP32, tag="drow")
            nc.gpsimd.partition_broadcast(drow[:], drow1[0:1, :])
            DROW[b] = drow

            # fold dinv[f]*m[f] into stored adjacency (per-partition scale)
            for et in range(ET):
                nc.vector.tensor_scalar_mul(
                    AH[b][et][:], AH[b][et][:], dm[:, et:et + 1]
                )

        def stages2to5(b):
            psp = PSP[b]
            # -- stage 2: outT[d,e] = sum_f X[f,d] * Ahat[f,e], * dinv[e] --
            for dt in range(KD):
                if dt % 2 == 0:
                    oft = oftp.tile([128, 2, E], FP8, tag=f"oft{dt // 2}")
                    OFT[b][dt // 2] = oft
                else:
                    oft = OFT[b][dt // 2]
                ps = psp.tile([128, E], FP32)
                for ec in range(FC):
                    for k in range(ET):
                        nc.tensor.matmul(
                            ps[:, ec * 512:(ec + 1) * 512],
                            XN[b][k][:, dt * 128:(dt + 1) * 128],
                            AH[b][k][:, ec * 512:(ec + 1) * 512],
                            start=(k == 0),
                            stop=(k == ET - 1),
                        )
                nc.vector.tensor_mul(
                    oft[:, dt % 2, :],
                    ps[:],
                    DROW[b][:],
                )

            # -- stage 3: HfT = relu(Wg.T @ outT + bg) --
            for ht in range(KD):
                if ht % 2 == 0:
                    hf = hftp.tile([128, 2, E], FP8, tag=f"hft{ht // 2}")
                    HFT[b][ht // 2] = hf
                else:
                    hf = HFT[b][ht // 2]
                ps = psp.tile([128, E], FP32)
                for ec in range(FC):
                    for k in range(KP):
                        nc.tensor.matmul(
                            ps[:, ec * 512:(ec + 1) * 512],
                            wg[k][:, :, ht * 128:(ht + 1) * 128],
                            OFT[b][k][:, :, ec * 512:(ec + 1) * 512],
                            start=(k == 0),
                            stop=(k == KP - 1),
                            perf_mode=mybir.MatmulPerfMode.DoubleRow,
                        )
                nc.scalar.activation(
                    hf[:, ht % 2, :],
                    ps[:],
                    AF.Relu,
                    bias=bg[:, ht:ht + 1],
                )

            # -- stage 4: PT = Wp.T @ HfT + bp (evicted as fp8 pair tiles) --
            for ht in range(KD):
                if ht % 2 == 0:
                    pt = ptp.tile([128, 2, E], FP8, tag=f"pt{ht // 2}")
                    PT[b][ht // 2] = pt
                else:
                    pt = PT[b][ht // 2]
                ps = psp.tile([128, E], FP32)
                for ec in range(FC):
                    for k in range(KP):
                        nc.tensor.matmul(
                            ps[:, ec * 512:(ec + 1) * 512],
                            wp[k][:, :, ht * 128:(ht + 1) * 128],
                            HFT[b][k][:, :, ec * 512:(ec + 1) * 512],
                            start=(k == 0),
                            stop=(k == KP - 1),
                            perf_mode=mybir.MatmulPerfMode.DoubleRow,
                        )
                nc.vector.tensor_scalar_add(
                    pt[:, ht % 2, :],
                    ps[:],
                    bp[:, ht:ht + 1],
                )

            # -- stage 5: Aout = sigmoid(PT.T @ PT) * pair --
            for et in range(ET):
                ost = ostp.tile([128, E], FP32)
                ps = psp.tile([128, E], FP32)
                for fc in range(FC):
                    for k in range(KP):
                        nc.tensor.matmul(
                            ps[:, fc * 512:(fc + 1) * 512],
                            PT[b][k][:, :, et * 128:(et + 1) * 128],
                            PT[b][k][:, :, fc * 512:(fc + 1) * 512],
                            start=(k == 0),
                            stop=(k == KP - 1),
                            perf_mode=mybir.MatmulPerfMode.DoubleRow,
                        )
                nc.scalar.activation(ost[:], ps[:], AF.Sigmoid)
                nc.vector.scalar_tensor_tensor(
                    out=ost[:],
                    in0=ost[:],
                    scalar=MCOL[b][:, et:et + 1],
                    in1=MROW[b][:],
                    op0=AL.mult,
                    op1=AL.mult,
                )
                nc.sync.dma_start(
                    out_d[b, et * 128:(et + 1) * 128, :], ost[:]
                )

        def load_stage2_inputs(b):
            # separate DMA queue (gpsimd/SWDGE) so these don't delay the
            # stage-1 XP loads on the sync queue
            for k in range(ET):
                t = xnp.tile([128, D], BF16, tag=f"xn{k}")
                nc.gpsimd.dma_start(t[:], xn_d[b, k * 128:(k + 1) * 128, :])
                XN[b][k] = t

        for b in range(BL):
            load_inputs(b)
        for b in range(BL):
            load_stage2_inputs(b)

        # ---- weights / constants (loaded after X: first use is stage 3) ----
        wg = []
        wp = []
        for k in range(KP):
            t = wpool.tile([128, 2, H], FP8, tag=f"wg{k}")
            nc.gpsimd.dma_start(t[:], wg_d[k, :, :, :])
            wg.append(t)
        for k in range(KP):
            t = wpool.tile([128, 2, H], FP8, tag=f"wp{k}")
            nc.gpsimd.dma_start(t[:], wp_d[k, :, :, :])
            wp.append(t)
        bg = wpool.tile([128, KD], FP32, tag="bg")
        nc.sync.dma_start(bg[:], bg_d[:])
        bp = wpool.tile([128, KD], FP32, tag="bp")
        nc.sync.dma_start(bp[:], bp_d[:])

        for _ in range(loops):
            PSP[0], PSP[1] = psA, psB
            for b in range(BL):
                stage1(b)
            for b in range(BL):
                stages2to5(b)

    nc.compile()
    _cached_nc[loops] = nc
    return nc


def make_in_maps(X, mask, W_gcn, b_gcn, W_proj, b_proj):
    bf = ml_dtypes.bfloat16
    f8 = mybir.dt.np(FP8)
    X = np.ascontiguousarray(np.asarray(X, dtype=np.float32))
    m = np.asarray(mask).astype(np.float32)
    wgT = np.asarray(W_gcn, np.float32).T
    wpT = np.asarray(W_proj, np.float32).T
    wg = np.ascontiguousarray(
        wgT.reshape(KP, 2, 128, H).transpose(0, 2, 1, 3)).astype(f8)
    wp = np.ascontiguousarray(
        wpT.reshape(KP, 2, 128, H).transpose(0, 2, 1, 3)).astype(f8)
    bg = np.ascontiguousarray(np.asarray(b_gcn, np.float32).reshape(KD, 128).T)
    bp = np.ascontiguousarray(np.asarray(b_proj, np.float32).reshape(KD, 128).T)
    eye = np.eye(128, dtype=bf)
    in_maps = []
    for c in range(NCORES):
        sl = slice(c * BL, (c + 1) * BL)
        Xc = X[sl]
        mc = m[sl]
        in_maps.append({
            "XP": np.ascontiguousarray(
                Xc.transpose(0, 2, 1).reshape(BL, KP, 2, 128, E)
                .transpose(0, 1, 3, 2, 4)
            ).astype(f8),
            "XN": Xc.astype(bf),
            "WG": wg,
            "WP": wp,
            "BG": bg,
            "BP": bp,
            "MROW": mc.reshape(BL, 1, E).astype(bf),
            "MCOL": np.ascontiguousarray(
                mc.reshape(BL, ET, 128).transpose(0, 2, 1)
            ),
            "EYE": eye,
        })
    return in_maps


def kernel(X, mask, W_gcn, b_gcn, W_proj, b_proj):
    nc = _build()
    in_maps = make_in_maps(X, mask, W_gcn, b_gcn, W_proj, b_proj)
    res = run_bass_kernel_spmd(nc, in_maps, list(range(NCORES)))
    out = np.concatenate([r["OUT"] for r in res.results], axis=0)
    return np.ascontiguousarray(out.astype(np.float32))


# revision 27
# speedup vs baseline: 41.9165x; 26.5828x over previous
"""GCN decoder kernel for Trainium2 (8 NeuronCores, data-parallel over batch).

Per batch element b (E=1024 nodes, D=H=768):
  S    = X @ X^T                          (PE, bf16 in / fp32 accum)
  Ahat = sigmoid(S)*m[col] (+I on diag)   (ACT sigmoid + fused DVE mask;
                                           row sums captured by the same op)
  deg  = m * rowsum(Ahat)                 -> dinv = (max(deg,1e-6))^-0.5
  Ahat *= (dinv*m)  per-partition         (folds the left D^-1/2 factor and
                                           the pair mask; A's symmetry makes
                                           the stored [e,f] tile the [f,e] rhs)
  outT = X_lhs.T @ Ahat, scaled by dinv along free dim on PSUM eviction
  HfT  = relu(Wg^T.T @ outT + bg)         (ACT bias+relu eviction)
  PT   = Wp^T.T @ HfT + bp                (DVE bias eviction)
  S2   = PT.T @ PT ; out = sigmoid(S2) * m[row] * m[col]  (fused DVE mask)

dinv also needs to exist as a broadcast row along the free dim; that
transpose is done with a DRAM bounce (contiguous write, strided read) so no
compute engine sits on the critical path, then gpsimd.partition_broadcast.

The gt pool holds one slot per stage-1 chunk (17 > 16), so a whole batch's
sigmoid chain can drain PSUM without waiting on any DVE slot release; that
makes it safe to emit batch 1's stage 1 right after batch 0's (keeping the
PE busy through batch 0's degree chain) without the DVE FIFO priority
inversion deadlocking the schedule.

Sharding: batch 16 -> 2 per core across 8 cores; weights replicated.
"""

import sys

if "/opt/trn_rl_repo" not in sys.path:
    sys.path.insert(0, "/opt/trn_rl_repo")

from contextlib import ExitStack

import numpy as np
import ml_dtypes

import concourse.bass as bass
import concourse.tile as tile
from concourse import bacc, mybir
from concourse.bass_utils import run_bass_kernel_spmd

B, E, D, H = 16, 1024, 768, 768
NCORES = 8
BL = B // NCORES          # batch elements per core
ET = E // 128             # 8 e/f tiles
KD = D // 128             # 6 d/h tiles
FC = E // 512             # 2 moving chunks of 512

FP32 = mybir.dt.float32
I32 = mybir.dt.int32
BF16 = mybir.dt.bfloat16
FP8 = mybir.dt.float8e4
KP = D // 256             # 3 fp8 DoubleRow contraction pair-tiles
AL = mybir.AluOpType
AF = mybir.ActivationFunctionType

_cached_nc = {}


def _build(loops=1):
    if loops in _cached_nc:
        return _cached_nc[loops]

    nc = bacc.Bacc("TRN2", target_bir_lowering=False, debug=False)

    xp_d = nc.dram_tensor("XP", [BL, KP, 128, 2, E], FP8, kind="ExternalInput")
    xn_d = nc.dram_tensor("XN", [BL, E, D], BF16, kind="ExternalInput")
    wg_d = nc.dram_tensor("WG", [KP, 128, 2, H], FP8, kind="ExternalInput")
    wp_d = nc.dram_tensor("WP", [KP, 128, 2, H], FP8, kind="ExternalInput")
    bg_d = nc.dram_tensor("BG", [128, KD], FP32, kind="ExternalInput")
    bp_d = nc.dram_tensor("BP", [128, KD], FP32, kind="ExternalInput")
    mrow_d = nc.dram_tensor("MROW", [BL, 1, E], BF16, kind="ExternalInput")
    mcol_d = nc.dram_tensor("MCOL", [BL, 128, ET], FP32, kind="ExternalInput")
    eye_d = nc.dram_tensor("EYE", [128, 128], BF16, kind="ExternalInput")
    out_d = nc.dram_tensor("OUT", [BL, E, E], FP32, kind="ExternalOutput")

    with tile.TileContext(nc) as tc, ExitStack() as ctx:
        ep = ctx.enter_context
        wpool = ep(tc.tile_pool(name="wpool", bufs=1))
        xtp = ep(tc.tile_pool(name="xt", bufs=2))
        xnp = ep(tc.tile_pool(name="xn", bufs=2))
        ahp = ep(tc.tile_pool(name="ahat", bufs=2))
        rows = ep(tc.tile_pool(name="rows", bufs=2))
        stat = ep(tc.tile_pool(name="stat", bufs=2))
        oftp = ep(tc.tile_pool(name="oft", bufs=2))
        hftp = ep(tc.tile_pool(name="hft", bufs=2))
        ptp = ep(tc.tile_pool(name="ptp", bufs=2))
        rowtmp = ep(tc.tile_pool(name="rowtmp", bufs=1))
        drowp = ep(tc.tile_pool(name="drowp", bufs=2))
        gtp = ep(tc.tile_pool(name="gt", bufs=9))
        ostp = ep(tc.tile_pool(name="ost", bufs=4))
        psA = ep(tc.tile_pool(name="psumA", bufs=2, space="PSUM"))
        psB = ep(tc.tile_pool(name="psumB", bufs=2, space="PSUM"))
        dscr = ep(tc.tile_pool(name="dscr", bufs=2, space="DRAM"))

        # ---- per-batch inputs ----
        XT = [[None] * KP for _ in range(BL)]
        XN = [[None] * ET for _ in range(BL)]
        MROW = [None] * BL
        MCOL = [None] * BL
        AH = [[None] * ET for _ in range(BL)]
        DROW = [None] * BL
        OFT = [[None] * KP for _ in range(BL)]
        HFT = [[None] * KP for _ in range(BL)]
        PT = [[None] * KP for _ in range(BL)]

        eye = wpool.tile([128, 128], BF16, tag="eye")
        nc.sync.dma_start(eye[:], eye_d[:])

        def load_inputs(b):
            for k in range(KP):
                t = xtp.tile([128, 2, E], FP8, tag=f"xp{k}")
                nc.sync.dma_start(t[:, :, 0:512], xp_d[b, k, :, :, 0:512])
                nc.sync.dma_start(t[:, :, 512:E], xp_d[b, k, :, :, 512:E])
                XT[b][k] = t
            r1 = rows.tile([1, E], BF16, tag="row1")
            nc.sync.dma_start(r1[:], mrow_d[b, :, :])
            mrow = rows.tile([128, E], BF16, tag="mrow")
            nc.gpsimd.partition_broadcast(mrow[:], r1[0:1, :])
            MROW[b] = mrow
            mc = rows.tile([128, ET], FP32, tag="mcol")
            nc.sync.dma_start(mc[:], mcol_d[b, :, :])
            MCOL[b] = mc

        PSP = [None, None]

        def stage1(b):
            psp = PSP[b]
            # S = X X^T; Ahat = sigmoid(S) * m[row-free] (+ I); rowsums.
            rs_a = stat.tile([128, ET], FP32, tag="rsa")
            for et in range(ET):
                ah = ahp.tile([128, E], BF16, tag=f"ah{et}")
                AH[b][et] = ah
                ps = psp.tile([128, E], FP32)
                for fc in range(FC):
                    for k in range(KP):
                        nc.tensor.matmul(
                            ps[:, fc * 512:(fc + 1) * 512],
                            XT[b][k][:, :, et * 128:(et + 1) * 128],
                            XT[b][k][:, :, fc * 512:(fc + 1) * 512],
                            start=(k == 0),
                            stop=(k == KP - 1),
                            perf_mode=mybir.MatmulPerfMode.DoubleRow,
                        )
                g = gtp.tile([128, E], BF16)
                nc.scalar.activation(g[:], ps[:], AF.Sigmoid)
                nc.vector.scalar_tensor_tensor(
                    out=ah[:],
                    in0=g[:],
                    scalar=1.0,
                    in1=MROW[b][:],
                    op0=AL.mult,
                    op1=AL.mult,
                    accum_out=rs_a[:, et:et + 1],
                )
                # self-loop: +1 on the diagonal 128-block of this e-tile
                nc.vector.tensor_add(
                    ah[:, et * 128:(et + 1) * 128],
                    ah[:, et * 128:(et + 1) * 128],
                    eye[:],
                )

            # degree -> dinv chain (per-partition [128, 8] layout)
            deg = stat.tile([128, ET], FP32, tag="deg")
            nc.vector.scalar_tensor_tensor(
                out=deg[:], in0=rs_a[:], scalar=1.0, in1=MCOL[b][:],
                op0=AL.add, op1=AL.mult,
            )
            nc.vector.tensor_scalar_max(deg[:], deg[:], 1e-6)
            sq = stat.tile([128, ET], FP32, tag="sq")
            nc.scalar.sqrt(sq[:], deg[:])
            dinv = stat.tile([128, ET], FP32, tag="dinv")
            nc.vector.reciprocal(dinv[:], sq[:])
            dm = stat.tile([128, ET], FP32, tag="dm")
            nc.vector.tensor_mul(dm[:], dinv[:], MCOL[b][:])

            # dinv as a broadcast row: DRAM bounce does the [128,8]->[1,1024]
            # transpose (contiguous write, strided read) off every engine.
            dsc = dscr.tile([128, ET], FP32, tag="dsc")
            nc.sync.dma_start(dsc[:], dinv[:])
            drow1 = rowtmp.tile([1, E], FP32, tag="drow1")
            nc.sync.dma_start(drow1[0:1, :], dsc[:, :].rearrange("p t -> t p"))
            drow = drowp.tile([128, E], FP32, tag="drow")
            nc.gpsimd.partition_broadcast(drow[:], drow1[0:1, :])
            DROW[b] = drow

            # fold dinv[f]*m[f] into stored adjacency (per-partition scale)
            for et in range(ET):
                nc.vector.tensor_scalar_mul(
                    AH[b][et][:], AH[b][et][:], dm[:, et:et + 1]
                )

        def stages2to5(b):
            psp = PSP[b]
            # -- stage 2: outT[d,e] = sum_f X[f,d] * Ahat[f,e], * dinv[e] --
            for dt in range(KD):
                if dt % 2 == 0:
                    oft = oftp.tile([128, 2, E], FP8, tag=f"oft{dt // 2}")
                    OFT[b][dt // 2] = oft
                else:
                    oft = OFT[b][dt // 2]
                ps = psp.tile([128, E], FP32)
                for ec in range(FC):
                    for k in range(ET):
                        nc.tensor.matmul(
                            ps[:, ec * 512:(ec + 1) * 512],
                            XN[b][k][:, dt * 128:(dt + 1) * 128],
                            AH[b][k][:, ec * 512:(ec + 1) * 512],
                            start=(k == 0),
                            stop=(k == ET - 1),
                        )
                nc.vector.tensor_mul(
                    oft[:, dt % 2, :],
                    ps[:],
                    DROW[b][:],
                )

            # -- stage 3: HfT = relu(Wg.T @ outT + bg) --
            for ht in range(KD):
                if ht % 2 == 0:
                    hf = hftp.tile([128, 2, E], FP8, tag=f"hft{ht // 2}")
                    HFT[b][ht // 2] = hf
                else:
                    hf = HFT[b][ht // 2]
                ps = psp.tile([128, E], FP32)
                for ec in range(FC):
                    for k in range(KP):
                        nc.tensor.matmul(
                            ps[:, ec * 512:(ec + 1) * 512],
                            wg[k][:, :, ht * 128:(ht + 1) * 128],
                            OFT[b][k][:, :, ec * 512:(ec + 1) * 512],
                            start=(k == 0),
                            stop=(k == KP - 1),
                            perf_mode=mybir.MatmulPerfMode.DoubleRow,
                        )
                nc.scalar.activation(
                    hf[:, ht % 2, :],
                    ps[:],
                    AF.Relu,
                    bias=bg[:, ht:ht + 1],
                )

            # -- stage 4: PT = Wp.T @ HfT + bp (evicted as fp8 pair tiles) --
            for ht in range(KD):
                if ht % 2 == 0:
                    pt = ptp.tile([128, 2, E], FP8, tag=f"pt{ht // 2}")
                    PT[b][ht // 2] = pt
                else:
                    pt = PT[b][ht // 2]
                ps = psp.tile([128, E], FP32)
                for ec in range(FC):
                    for k in range(KP):
                        nc.tensor.matmul(
                            ps[:, ec * 512:(ec + 1) * 512],
                            wp[k][:, :, ht * 128:(ht + 1) * 128],
                            HFT[b][k][:, :, ec * 512:(ec + 1) * 512],
                            start=(k == 0),
                            stop=(k == KP - 1),
                            perf_mode=mybir.MatmulPerfMode.DoubleRow,
                        )
                nc.vector.tensor_scalar_add(
                    pt[:, ht % 2, :],
                    ps[:],
                    bp[:, ht:ht + 1],
                )

            # -- stage 5: Aout = sigmoid(PT.T @ PT) * pair --
            for et in range(ET):
                ost = ostp.tile([128, E], FP32)
                ps = psp.tile([128, E], FP32)
                for fc in range(FC):
                    for k in range(KP):
                        nc.tensor.matmul(
                            ps[:, fc * 512:(fc + 1) * 512],
                            PT[b][k][:, :, et * 128:(et + 1) * 128],
                            PT[b][k][:, :, fc * 512:(fc + 1) * 512],
                            start=(k == 0),
                            stop=(k == KP - 1),
                            perf_mode=mybir.MatmulPerfMode.DoubleRow,
                        )
                nc.scalar.activation(ost[:], ps[:], AF.Sigmoid)
                nc.vector.scalar_tensor_tensor(
                    out=ost[:],
                    in0=ost[:],
                    scalar=MCOL[b][:, et:et + 1],
                    in1=MROW[b][:],
                    op0=AL.mult,
                    op1=AL.mult,
                )
                nc.sync.dma_start(
                    out_d[b, et * 128:(et + 1) * 128, :], ost[:]
                )

        def load_stage2_inputs(b):
            # separate DMA queue (gpsimd/SWDGE) so these don't delay the
            # stage-1 XP loads on the sync queue
            for k in range(ET):
                t = xnp.tile([128, D], BF16, tag=f"xn{k}")
                nc.gpsimd.dma_start(t[:], xn_d[b, k * 128:(k + 1) * 128, :])
                XN[b][k] = t

        for b in range(BL):
            load_inputs(b)
        for b in range(BL):
            load_stage2_inputs(b)

        # ---- weights / constants (loaded after X: first use is stage 3) ----
        wg = []
        wp = []
        for k in range(KP):
            t = wpool.tile([128, 2, H], FP8, tag=f"wg{k}")
            nc.gpsimd.dma_start(t[:], wg_d[k, :, :, :])
            wg.append(t)
        for k in range(KP):
            t = wpool.tile([128, 2, H], FP8, tag=f"wp{k}")
            nc.gpsimd.dma_start(t[:], wp_d[k, :, :, :])
            wp.append(t)
        bg = wpool.tile([128, KD], FP32, tag="bg")
        nc.sync.dma_start(bg[:], bg_d[:])
        bp = wpool.tile([128, KD], FP32, tag="bp")
        nc.sync.dma_start(bp[:], bp_d[:])

        for _ in range(loops):
            PSP[0], PSP[1] = psA, psB
            for b in range(BL):
                stage1(b)
            for b in range(BL):
                stages2to5(b)

    nc.compile()
    _cached_nc[loops] = nc
    return nc


def make_in_maps(X, mask, W_gcn, b_gcn, W_proj, b_proj):
    bf = ml_dtypes.bfloat16
    f8 = mybir.dt.np(FP8)
    X = np.ascontiguousarray(np.asarray(X, dtype=np.float32))
    m = np.asarray(mask).astype(np.float32)
    wgT = np.asarray(W_gcn, np.float32).T
    wpT = np.asarray(W_proj, np.float32).T
    wg = np.ascontiguousarray(
        wgT.reshape(KP, 2, 128, H).transpose(0, 2, 1, 3)).astype(f8)
    wp = np.ascontiguousarray(
        wpT.reshape(KP, 2, 128, H).transpose(0, 2, 1, 3)).astype(f8)
    bg = np.ascontiguousarray(np.asarray(b_gcn, np.float32).reshape(KD, 128).T)
    bp = np.ascontiguousarray(np.asarray(b_proj, np.float32).reshape(KD, 128).T)
    eye = np.eye(128, dtype=bf)
    in_maps = []
    for c in range(NCORES):
        sl = slice(c * BL, (c + 1) * BL)
        Xc = X[sl]
        mc = m[sl]
        in_maps.append({
            "XP": np.ascontiguousarray(
                Xc.transpose(0, 2, 1).reshape(BL, KP, 2, 128, E)
                .transpose(0, 1, 3, 2, 4)
            ).astype(f8),
            "XN": Xc.astype(bf),
            "WG": wg,
            "WP": wp,
            "BG": bg,
            "BP": bp,
            "MROW": mc.reshape(BL, 1, E).astype(bf),
            "MCOL": np.ascontiguousarray(
                mc.reshape(BL, ET, 128).transpose(0, 2, 1)
            ),
            "EYE": eye,
        })
    return in_maps


def kernel(X, mask, W_gcn, b_gcn, W_proj, b_proj):
    nc = _build()
    in_maps = make_in_maps(X, mask, W_gcn, b_gcn, W_proj, b_proj)
    res = run_bass_kernel_spmd(nc, in_maps, list(range(NCORES)))
    out = np.concatenate([r["OUT"] for r in res.results], axis=0)
    return np.ascontiguousarray(out.astype(np.float32))
